# revision 10
# baseline (speedup 1.0000x reference)
"""VQ codebook (nn_Codebook_54752243089494) Trainium2 kernel.

kernel(**inputs) takes the FULL inputs (z [32,256,64,64] f32,
embedding [1024,256] f32, code_usage [1024] f32, code_avg [1024,256] f32)
and returns the reference 7-tuple. Tokens (B*H*W = 131072) are sharded
data-parallel across 8 NeuronCores (16384 tokens/core); the codebook is
replicated. Per-device segment sums are reduced on host before the
(tiny, [K]/[K,d]-sized) EMA update.

Per-core device program, per 128-token tile:
  scores = x.e                         (PE fp32, PSUM)
  negd   = scores - e_sq/2, rowmax     (DVE tensor_tensor_reduce)
  onehot = (negd == rowmax)  fp16      (DVE TTR)
  idx    = K - max(onehot * (K-iota))  (DVE TTR)
  qf     = embedding[idx]              (indirect-DMA row gather)
  ef, qz = PE transposes of z / qf     (+ ACT PSUM->SBUF copies)
  seg   += x_tok^T @ onehot            (PE fp16, PSUM accumulate)
"""

import sys

sys.path.insert(0, "/opt/trn_rl_repo")

import numpy as np

import concourse.bass as bass
import concourse.bacc as bacc
import concourse.mybir as mybir
import concourse.tile as tile
from concourse.bass_utils import run_bass_kernel_spmd
from concourse.masks import make_identity

F32 = mybir.dt.float32
F16 = mybir.dt.float16
I32 = mybir.dt.int32
U32 = mybir.dt.uint32
AX = mybir.AluOpType

DECAY = 0.99
EPS = 1e-07
N_CORES = 8


def build_vq_kernel(nb=4, s_tiles=32, K=1024, d=256):
    """Build the per-core Bass program. Full size: nb=4, s_tiles=32."""
    S = s_tiles * 128          # spatial positions per batch
    nt = nb * s_tiles          # token tiles per core
    assert d == 256 and K % 512 == 0

    nc = bacc.Bacc()
    z_d = nc.dram_tensor("z", [nb, d, S], F32, kind="ExternalInput")
    embT_d = nc.dram_tensor("embT", [d, K], F32, kind="ExternalInput")
    emb_d = nc.dram_tensor("emb", [K, d], F32, kind="ExternalInput")
    ef_d = nc.dram_tensor("ef", [nb * S, d], F32, kind="ExternalOutput")
    qf_d = nc.dram_tensor("qf", [nb * S, d], F32, kind="ExternalOutput")
    qz_d = nc.dram_tensor("qz", [nb, d, S], F32, kind="ExternalOutput")
    idx_d = nc.dram_tensor("idx", [nt, 128], I32, kind="ExternalOutput")
    seg_d = nc.dram_tensor("seg", [d, K], F32, kind="ExternalOutput")

    with tile.TileContext(nc) as tc:
        with (
            tc.tile_pool(name="const", bufs=1) as cpool,
            tc.tile_pool(name="zpool", bufs=1) as zpool,
            tc.tile_pool(name="work", bufs=2) as wpool,
            tc.tile_pool(name="psum", bufs=1, space="PSUM") as ppool,
            tc.tile_pool(name="tpsum", bufs=2, space="PSUM") as tpool,
        ):
            # ---------------- setup ----------------
            embT_sb = []
            for ch in range(2):
                t = cpool.tile([128, K], F32, tag=f"embT{ch}", name=f"embT{ch}")
                nc.sync.dma_start(out=t[:], in_=embT_d[ch * 128:(ch + 1) * 128, :])
                embT_sb.append(t)

            ident = cpool.tile([128, 128], F32, tag="ident")
            make_identity(nc, ident[:])

            ones_col = cpool.tile([128, 1], F32, tag="ones_col")
            nc.vector.memset(ones_col[:], 1.0)
            ones_row = cpool.tile([1, 128], F32, tag="ones_row")
            nc.vector.memset(ones_row[:], 1.0)

            # nhalf = -e_sq/2, e_sq = sum_c embT^2   (row vector [1, K])
            nhalf = cpool.tile([1, K], F32, tag="nhalf")
            sq = []
            for ch in range(2):
                s_t = wpool.tile([128, K], F32, tag="negd", name=f"sq{ch}")
                nc.vector.tensor_tensor(out=s_t[:], in0=embT_sb[ch][:],
                                        in1=embT_sb[ch][:], op=AX.mult)
                sq.append(s_t)
            esq_ps = ppool.tile([1, K], F32, tag="scores", space="PSUM")
            for half in range(K // 512):
                hs = slice(half * 512, half * 512 + 512)
                for ch in range(2):
                    nc.tensor.matmul(out=esq_ps[:, hs], lhsT=ones_col[:],
                                     rhs=sq[ch][:, hs],
                                     start=(ch == 0), stop=(ch == 1))
            nc.vector.tensor_scalar(out=nhalf[:], in0=esq_ps[:], scalar1=-0.5,
                                    scalar2=None, op0=AX.mult)

            # persistent accumulators
            idxacc = cpool.tile([128, 8 * nt], U32, tag="idxacc")
            seg_ps = []
            for ch in range(2):
                seg_t = ppool.tile([128, K], F32, tag=f"seg{ch}",
                                   name=f"seg{ch}", space="PSUM")
                seg_ps.append(seg_t)

            # ---------------- main loop ----------------
            zt = {}
            for b in range(nb):
                for ch in range(2):
                    t = zpool.tile([128, S], F32, tag=f"z{b}{ch}",
                                   name=f"z{b}{ch}")
                    nc.sync.dma_start(
                        out=t[:], in_=z_d[b, ch * 128:(ch + 1) * 128, :])
                    zt[(b, ch)] = t

                for s in range(s_tiles):
                    j = b * s_tiles + s
                    ts_ = slice(s * 128, (s + 1) * 128)

                    # transposed x (for ef out, segsum lhsT); also first
                    # PE reader of a fresh z tile
                    tp = tpool.tile([128, 4 * 128], F32, tag="tp",
                                    space="PSUM")
                    for ch in range(2):
                        nc.tensor.transpose(
                            out=tp[:, ch * 128:(ch + 1) * 128],
                            in_=zt[(b, ch)][:, ts_], identity=ident[:])

                    # scores = -esq/2 + x.e   [t, K]
                    sc_ps = ppool.tile([128, K], F32, tag="scores",
                                       space="PSUM")
                    for half in range(K // 512):
                        hs = slice(half * 512, half * 512 + 512)
                        nc.tensor.matmul(out=sc_ps[:, hs], lhsT=ones_row[:],
                                         rhs=nhalf[:, hs],
                                         start=True, stop=False)
                        for ch in range(2):
                            nc.tensor.matmul(out=sc_ps[:, hs],
                                             lhsT=zt[(b, ch)][:, ts_],
                                             rhs=embT_sb[ch][:, hs],
                                             start=False, stop=(ch == 1))

                    # ACT: PSUM -> SBUF
                    negd = wpool.tile([128, K], F32, tag="negd")
                    nc.scalar.copy(out=negd[:], in_=sc_ps[:])

                    # DVE: argmax + onehot
                    mx = wpool.tile([128, 8], F32, tag="mx")
                    nc.vector.max(out=mx[:], in_=negd[:])
                    nc.vector.max_index(out=idxacc[:, 8 * j:8 * j + 8],
                                        in_max=mx[:], in_values=negd[:])
                    onehot = wpool.tile([128, K], F16, tag="onehot")
                    nc.vector.tensor_scalar(out=onehot[:], in0=negd[:],
                                            scalar1=mx[:, 0:1], scalar2=None,
                                            op0=AX.is_equal)

                    # gather quantized rows (idx as int32 view of u32 col)
                    qf_sb = wpool.tile([128, d], F32, tag="qf")
                    nc.gpsimd.indirect_dma_start(
                        out=qf_sb[:], out_offset=None, in_=emb_d[:, :],
                        in_offset=bass.IndirectOffsetOnAxis(
                            ap=idxacc[:, 8 * j:8 * j + 1].bitcast(I32),
                            axis=0))
                    nc.sync.dma_start(out=qf_d[j * 128:(j + 1) * 128, :],
                                      in_=qf_sb[:])

                    for ch in range(2):
                        nc.tensor.transpose(
                            out=tp[:, 256 + ch * 128:256 + (ch + 1) * 128],
                            in_=qf_sb[:, ch * 128:(ch + 1) * 128],
                            identity=ident[:])

                    x_tok = wpool.tile([128, d], F32, tag="x_tok")
                    nc.scalar.copy(out=x_tok[:], in_=tp[:, 0:256])
                    x16 = wpool.tile([128, d], F16, tag="x16")
                    nc.scalar.copy(out=x16[:], in_=tp[:, 0:256])
                    qzm = wpool.tile([128, d], F32, tag="qzm")
                    nc.scalar.copy(out=qzm[:], in_=tp[:, 256:512])

                    nc.sync.dma_start(out=ef_d[j * 128:(j + 1) * 128, :],
                                      in_=x_tok[:])
                    for ch in range(2):
                        nc.sync.dma_start(
                            out=qz_d[b, ch * 128:(ch + 1) * 128, ts_],
                            in_=qzm[:, ch * 128:(ch + 1) * 128])

                    # seg[ch] += x16[:, ch].T @ onehot
                    for ch in range(2):
                        for half in range(K // 512):
                            hs = slice(half * 512, half * 512 + 512)
                            nc.tensor.matmul(
                                out=seg_ps[ch][:, hs],
                                lhsT=x16[:, ch * 128:(ch + 1) * 128],
                                rhs=onehot[:, hs],
                                start=(j == 0), stop=(j == nt - 1))

            # ---------------- epilogue ----------------
            for ch in range(2):
                seg_sb = wpool.tile([128, K], F32, tag="negd",
                                    name=f"seg_sb{ch}")
                nc.scalar.copy(out=seg_sb[:], in_=seg_ps[ch][:])
                nc.sync.dma_start(out=seg_d[ch * 128:(ch + 1) * 128, :],
                                  in_=seg_sb[:])

            # indices: u32 cols (stride 8) -> f32 -> transpose -> i32 rows
            idxf = wpool.tile([128, nt], F32, tag="idxf")
            nc.vector.tensor_copy(
                out=idxf[:],
                in_=idxacc[:].rearrange("p (t e) -> p t e", e=8)[:, :, 0])
            tpx = ppool.tile([128, K], F32, tag="scores", name="tpx",
                             space="PSUM")
            nc.tensor.transpose(out=tpx[:nt, 0:128], in_=idxf[:],
                                identity=ident[:])
            idxT = wpool.tile([nt, 128], I32, tag="idxT")
            nc.scalar.copy(out=idxT[:], in_=tpx[:nt, 0:128])
            nc.sync.dma_start(out=idx_d[:, :], in_=idxT[:])

    nc.compile()
    return nc


DECAY = 0.99
EPS = 1e-07


_NC_CACHE = {}


def _get_nc():
    if "nc" not in _NC_CACHE:
        _NC_CACHE["nc"] = build_vq_kernel()
    return _NC_CACHE["nc"]


def kernel(z, embedding, code_usage, code_avg, _trace=False):
    z = np.ascontiguousarray(np.asarray(z, dtype=np.float32))
    embedding = np.ascontiguousarray(np.asarray(embedding, dtype=np.float32))
    code_usage = np.asarray(code_usage, dtype=np.float32)
    code_avg = np.asarray(code_avg, dtype=np.float32)

    B, C, H, W = z.shape            # 32, 256, 64, 64
    K, d = embedding.shape          # 1024, 256
    nb = B // N_CORES               # 4 batches per core
    S = H * W                       # 4096

    embT = np.ascontiguousarray(embedding.T)
    z_r = z.reshape(B, C, S)
    in_maps = [
        {"z": np.ascontiguousarray(z_r[m * nb:(m + 1) * nb]),
         "embT": embT, "emb": embedding}
        for m in range(N_CORES)
    ]

    nc = _get_nc()
    res = run_bass_kernel_spmd(nc, in_maps, list(range(N_CORES)),
                               trace=_trace)
    _NC_CACHE["last_result"] = res
    outs = res.results

    N = B * S
    encoded_flat = np.concatenate([o["ef"] for o in outs], axis=0)
    quantized_flat = np.concatenate([o["qf"] for o in outs], axis=0)
    codebook_indices = np.concatenate(
        [o["idx"].reshape(-1) for o in outs]).astype(np.int32)
    quantized = np.concatenate(
        [o["qz"].reshape(nb, C, H, W) for o in outs], axis=0)

    seg = np.zeros((K, d), dtype=np.float32)
    for o in outs:
        seg += o["seg"].T
    counts = np.bincount(codebook_indices, minlength=K).astype(np.float32)

    new_code_usage = code_usage * np.float32(DECAY) + \
        counts * np.float32(1.0 - DECAY)
    n = np.sum(new_code_usage, dtype=np.float32)
    new_code_usage = (new_code_usage + np.float32(EPS)) / \
        (n + np.float32(K) * np.float32(EPS)) * n
    new_code_avg = code_avg * np.float32(DECAY) + \
        np.float32(1.0 - DECAY) * seg
    new_embedding = new_code_avg / new_code_usage[:, None]

    return (encoded_flat, quantized_flat, codebook_indices, quantized,
            new_embedding, new_code_usage, new_code_avg)


# revision 12
# speedup vs baseline: 1.1428x; 1.1428x over previous
"""VQ codebook (nn_Codebook_54752243089494) Trainium2 kernel.

kernel(**inputs) takes the FULL inputs (z [32,256,64,64] f32,
embedding [1024,256] f32, code_usage [1024] f32, code_avg [1024,256] f32)
and returns the reference 7-tuple. Tokens (B*H*W = 131072) are sharded
data-parallel across 8 NeuronCores (16384 tokens/core); the codebook is
replicated. Per-device segment sums are reduced on host before the
(tiny, [K]/[K,d]-sized) EMA update.

Per-core device program, per 128-token tile:
  scores = x.e                         (PE fp32, PSUM)
  negd   = scores - e_sq/2, rowmax     (DVE tensor_tensor_reduce)
  onehot = (negd == rowmax)  fp16      (DVE TTR)
  idx    = K - max(onehot * (K-iota))  (DVE TTR)
  qf     = embedding[idx]              (indirect-DMA row gather)
  ef, qz = PE transposes of z / qf     (+ ACT PSUM->SBUF copies)
  seg   += x_tok^T @ onehot            (PE fp16, PSUM accumulate)
"""

import sys

sys.path.insert(0, "/opt/trn_rl_repo")

import numpy as np

import concourse.bass as bass
import concourse.bacc as bacc
import concourse.mybir as mybir
import concourse.tile as tile
from concourse.bass_utils import run_bass_kernel_spmd
from concourse.masks import make_identity

F32 = mybir.dt.float32
F16 = mybir.dt.float16
I32 = mybir.dt.int32
U32 = mybir.dt.uint32
AX = mybir.AluOpType

DECAY = 0.99
EPS = 1e-07
N_CORES = 8


BLK = 16      # seg-burst block (tiles)
GB = 4        # gather batch (tiles)


def build_vq_kernel(nb=4, s_tiles=32, K=1024, d=256):
    """Build the per-core Bass program. Full size: nb=4, s_tiles=32."""
    S = s_tiles * 128          # spatial positions per batch
    nt = nb * s_tiles          # token tiles per core
    assert d == 256 and K % 512 == 0
    assert s_tiles % GB == 0 or s_tiles < GB

    nc = bacc.Bacc()
    z_d = nc.dram_tensor("z", [nb, d, S], F32, kind="ExternalInput")
    embT_d = nc.dram_tensor("embT", [d, K], F32, kind="ExternalInput")
    emb_d = nc.dram_tensor("emb", [K, d], F32, kind="ExternalInput")
    ef_d = nc.dram_tensor("ef", [nb * S, d], F32, kind="ExternalOutput")
    qf_d = nc.dram_tensor("qf", [nb * S, d], F32, kind="ExternalOutput")
    qz_d = nc.dram_tensor("qz", [nb, d, S], F32, kind="ExternalOutput")
    idx_d = nc.dram_tensor("idx", [nt, 128], I32, kind="ExternalOutput")
    seg_d = nc.dram_tensor("seg", [d, K], F32, kind="ExternalOutput")

    blk = min(BLK, s_tiles)
    gb = min(GB, s_tiles)

    with tile.TileContext(nc) as tc:
        with (
            tc.tile_pool(name="const", bufs=1) as cpool,
            tc.tile_pool(name="zpool", bufs=1) as zpool,
            tc.tile_pool(name="work", bufs=2) as wpool,
            tc.tile_pool(name="hold", bufs=blk + 2) as hpool,
            tc.tile_pool(name="psum", bufs=2, space="PSUM") as ppool,
            tc.tile_pool(name="segp", bufs=1, space="PSUM") as spool,
            tc.tile_pool(name="tpsum", bufs=2, space="PSUM") as tpool,
        ):
            # ---------------- setup ----------------
            embT_sb = []
            for ch in range(2):
                t = cpool.tile([128, K], F32, tag=f"embT{ch}", name=f"embT{ch}")
                nc.sync.dma_start(out=t[:], in_=embT_d[ch * 128:(ch + 1) * 128, :])
                embT_sb.append(t)

            ident = cpool.tile([128, 128], F32, tag="ident")
            make_identity(nc, ident[:])

            ones_col = cpool.tile([128, 1], F32, tag="ones_col")
            nc.vector.memset(ones_col[:], 1.0)
            ones_row = cpool.tile([1, 128], F32, tag="ones_row")
            nc.vector.memset(ones_row[:], 1.0)
            ones8 = cpool.tile([128, 8], F16, tag="ones8")
            nc.vector.memset(ones8[:], 1.0)

            # nhalf = -e_sq/2, e_sq = sum_c embT^2   (row vector [1, K])
            nhalf = cpool.tile([1, K], F32, tag="nhalf")
            sq = []
            for ch in range(2):
                s_t = wpool.tile([128, K], F32, tag="negd", name=f"sq{ch}")
                nc.vector.tensor_tensor(out=s_t[:], in0=embT_sb[ch][:],
                                        in1=embT_sb[ch][:], op=AX.mult)
                sq.append(s_t)
            esq_ps = ppool.tile([1, K], F32, tag="scores", space="PSUM")
            for half in range(K // 512):
                hs = slice(half * 512, half * 512 + 512)
                for ch in range(2):
                    nc.tensor.matmul(out=esq_ps[:, hs], lhsT=ones_col[:],
                                     rhs=sq[ch][:, hs],
                                     start=(ch == 0), stop=(ch == 1))
            nc.vector.tensor_scalar(out=nhalf[:], in0=esq_ps[:], scalar1=-0.5,
                                    scalar2=None, op0=AX.mult)

            # persistent accumulators
            idxacc = cpool.tile([128, 8 * nt], U32, tag="idxacc")
            gidx = cpool.tile([128, nt], I32, tag="gidx")
            seg_sb = []
            for ch in range(2):
                t = cpool.tile([128, K], F32, tag=f"segsb{ch}",
                               name=f"segsb{ch}")
                nc.vector.memset(t[:], 0.0)
                seg_sb.append(t)

            # ---------------- main loop ----------------
            zt = {}
            for b in range(nb):
                for ch in range(2):
                    t = zpool.tile([128, S], F32, tag=f"z{b % 2}{ch}",
                                   name=f"z{b}{ch}")
                    nc.sync.dma_start(
                        out=t[:], in_=z_d[b, ch * 128:(ch + 1) * 128, :])
                    zt[(b, ch)] = t

                for s0 in range(0, s_tiles, blk):
                    nblk = min(blk, s_tiles - s0)
                    ohs, x16s = [], []
                    for s in range(s0, s0 + nblk):
                        j = b * s_tiles + s
                        ts_ = slice(s * 128, (s + 1) * 128)

                        # transposed x -> [x_tok | qz] staging
                        tp = tpool.tile([128, 256], F32, tag="tp",
                                        space="PSUM")
                        for ch in range(2):
                            nc.tensor.transpose(
                                out=tp[:, ch * 128:(ch + 1) * 128],
                                in_=zt[(b, ch)][:, ts_], identity=ident[:])

                        # scores = -esq/2 + x.e   [t, K]
                        sc_ps = ppool.tile([128, K], F32, tag="scores",
                                           space="PSUM")
                        for half in range(K // 512):
                            hs = slice(half * 512, half * 512 + 512)
                            nc.tensor.matmul(out=sc_ps[:, hs],
                                             lhsT=ones_row[:],
                                             rhs=nhalf[:, hs],
                                             start=True, stop=False)
                            for ch in range(2):
                                nc.tensor.matmul(out=sc_ps[:, hs],
                                                 lhsT=zt[(b, ch)][:, ts_],
                                                 rhs=embT_sb[ch][:, hs],
                                                 start=False, stop=(ch == 1))

                        negd = wpool.tile([128, K], F32, tag="negd")
                        nc.scalar.copy(out=negd[:], in_=sc_ps[:])

                        mx = wpool.tile([128, 8], F32, tag="mx")
                        nc.vector.max(out=mx[:], in_=negd[:])
                        onehot = hpool.tile([128, K], F16, tag="onehot")
                        nc.vector.tensor_scalar(out=onehot[:], in0=negd[:],
                                                scalar1=mx[:, 0:1],
                                                scalar2=None, op0=AX.is_equal)
                        nc.vector.max_index(out=idxacc[:, 8 * j:8 * j + 8],
                                            in_max=ones8[:],
                                            in_values=onehot[:])
                        nc.vector.tensor_copy(
                            out=gidx[:, j:j + 1],
                            in_=idxacc[:, 8 * j:8 * j + 1].bitcast(I32))
                        ohs.append(onehot)

                        # gather + quantized outputs
                        qf_sb = wpool.tile([128, d], F32, tag="qf")
                        nc.gpsimd.indirect_dma_start(
                            out=qf_sb[:], out_offset=None, in_=emb_d[:, :],
                            in_offset=bass.IndirectOffsetOnAxis(
                                ap=gidx[:, j:j + 1], axis=0))
                        nc.sync.dma_start(
                            out=qf_d[j * 128:(j + 1) * 128, :], in_=qf_sb[:])
                        tpq = tpool.tile([128, 256], F32, tag="tp",
                                         name="tpq", space="PSUM")
                        for ch in range(2):
                            nc.tensor.transpose(
                                out=tpq[:, ch * 128:(ch + 1) * 128],
                                in_=qf_sb[:, ch * 128:(ch + 1) * 128],
                                identity=ident[:])
                        qzm = wpool.tile([128, d], F32, tag="qzm")
                        nc.scalar.copy(out=qzm[:], in_=tpq[:, 0:256])
                        for ch in range(2):
                            nc.sync.dma_start(
                                out=qz_d[b, ch * 128:(ch + 1) * 128, ts_],
                                in_=qzm[:, ch * 128:(ch + 1) * 128])

                        x_tok = wpool.tile([128, d], F32, tag="x_tok")
                        nc.scalar.copy(out=x_tok[:], in_=tp[:, 0:256])
                        x16 = hpool.tile([128, d], F16, tag="x16")
                        nc.scalar.copy(out=x16[:], in_=tp[:, 0:256])
                        nc.sync.dma_start(out=ef_d[j * 128:(j + 1) * 128, :],
                                          in_=x_tok[:])
                        x16s.append(x16)

                    # seg burst: one c-chunk at a time through 2 PSUM banks
                    for ch in range(2):
                        sp = spool.tile([128, K], F32, tag="segps",
                                        space="PSUM")
                        for i in range(nblk):
                            for half in range(K // 512):
                                hs = slice(half * 512, half * 512 + 512)
                                nc.tensor.matmul(
                                    out=sp[:, hs],
                                    lhsT=x16s[i][:, ch * 128:(ch + 1) * 128],
                                    rhs=ohs[i][:, hs],
                                    start=(i == 0), stop=(i == nblk - 1))
                        nc.vector.tensor_tensor(out=seg_sb[ch][:],
                                                in0=seg_sb[ch][:],
                                                in1=sp[:], op=AX.add)

            # ---------------- epilogue ----------------
            for ch in range(2):
                nc.sync.dma_start(out=seg_d[ch * 128:(ch + 1) * 128, :],
                                  in_=seg_sb[ch][:])

            # indices: u32 cols (stride 8) -> f32 -> transpose -> i32 rows
            idxf = wpool.tile([128, nt], F32, tag="idxf")
            nc.vector.tensor_copy(out=idxf[:], in_=gidx[:])
            tpx = ppool.tile([128, K], F32, tag="scores", name="tpx",
                             space="PSUM")
            nc.tensor.transpose(out=tpx[:nt, 0:128], in_=idxf[:],
                                identity=ident[:])
            idxT = wpool.tile([nt, 128], I32, tag="idxT")
            nc.scalar.copy(out=idxT[:], in_=tpx[:nt, 0:128])
            nc.sync.dma_start(out=idx_d[:, :], in_=idxT[:])

    nc.compile()
    return nc


DECAY = 0.99
EPS = 1e-07


_NC_CACHE = {}


def _get_nc():
    if "nc" not in _NC_CACHE:
        _NC_CACHE["nc"] = build_vq_kernel()
    return _NC_CACHE["nc"]


def kernel(z, embedding, code_usage, code_avg, _trace=False):
    z = np.ascontiguousarray(np.asarray(z, dtype=np.float32))
    embedding = np.ascontiguousarray(np.asarray(embedding, dtype=np.float32))
    code_usage = np.asarray(code_usage, dtype=np.float32)
    code_avg = np.asarray(code_avg, dtype=np.float32)

    B, C, H, W = z.shape            # 32, 256, 64, 64
    K, d = embedding.shape          # 1024, 256
    nb = B // N_CORES               # 4 batches per core
    S = H * W                       # 4096

    embT = np.ascontiguousarray(embedding.T)
    z_r = z.reshape(B, C, S)
    in_maps = [
        {"z": np.ascontiguousarray(z_r[m * nb:(m + 1) * nb]),
         "embT": embT, "emb": embedding}
        for m in range(N_CORES)
    ]

    nc = _get_nc()
    res = run_bass_kernel_spmd(nc, in_maps, list(range(N_CORES)),
                               trace=_trace)
    _NC_CACHE["last_result"] = res
    outs = res.results

    N = B * S
    encoded_flat = np.concatenate([o["ef"] for o in outs], axis=0)
    quantized_flat = np.concatenate([o["qf"] for o in outs], axis=0)
    codebook_indices = np.concatenate(
        [o["idx"].reshape(-1) for o in outs]).astype(np.int32)
    quantized = np.concatenate(
        [o["qz"].reshape(nb, C, H, W) for o in outs], axis=0)

    seg = np.zeros((K, d), dtype=np.float32)
    for o in outs:
        seg += o["seg"].T
    counts = np.bincount(codebook_indices, minlength=K).astype(np.float32)

    new_code_usage = code_usage * np.float32(DECAY) + \
        counts * np.float32(1.0 - DECAY)
    n = np.sum(new_code_usage, dtype=np.float32)
    new_code_usage = (new_code_usage + np.float32(EPS)) / \
        (n + np.float32(K) * np.float32(EPS)) * n
    new_code_avg = code_avg * np.float32(DECAY) + \
        np.float32(1.0 - DECAY) * seg
    new_embedding = new_code_avg / new_code_usage[:, None]

    return (encoded_flat, quantized_flat, codebook_indices, quantized,
            new_embedding, new_code_usage, new_code_avg)
